# revision 1
# baseline (speedup 1.0000x reference)
"""2-layer GCN (GCNConv x2 + log_softmax) on 8 Trainium2 NeuronCores.

Strategy (graph/data parallel per sharding hint):
  - Host: degree-sorted node partitioning across 8 cores (balances edges and
    makes ELL tiles degree-uniform), edge lists bucketed by dst owner, packed
    into per-128-dst-node-tile ELL format (slot-major), int32 indices.
    Host computes deg^-1/2 from edge_index only (graph preprocessing).
  - NEFF1: per core, hs1 = (x_shard @ W1) * dis_shard  -> [16, S] transposed.
  - host: assemble full hs1 table (layout change only).
  - NEFF2: per core, ELL gather-aggregate over its dst shard, then
    out1 = relu(agg * dis + b1);  hs2 = (out1 * dis) @ W2 -> [2, S].
  - host: assemble full hs2 table.
  - NEFF3: per core, ELL gather-aggregate (8B rows), out = log_softmax(
    agg * dis + b2) -> [S, 2]. Host unpermutes rows.

Normalization trick: norm = dis[src]*dis[dst] factorizes, so we pre-scale the
message table by dis (producer side) and post-scale the aggregate by dis
(consumer side); no per-edge scaling needed.
"""

import math
import sys

import numpy as np

sys.path.insert(0, "/opt/trn_rl_repo")

from contextlib import ExitStack

import concourse.bacc as bacc
import concourse.tile as tile
from concourse import bass, mybir
from concourse.bass_utils import run_bass_kernel_spmd
from concourse.masks import make_identity

N_NODES = 100000
N_CORES = 8
P = 128
SHARD = 12544  # 98 * 128, padded per-core shard size
N_TILES = SHARD // P  # 98
F_IN, HID, OUT = 128, 16, 2
TABLE_ROWS = N_CORES * SHARD  # 100352
PAD_ROW = 12500  # core 0's first zero pad slot -> global row 12500 is zeros

_CACHE = {}


# ----------------------------------------------------------------- host prep
def _preprocess(edge_index):
    src = edge_index[0].astype(np.int64)
    dst = edge_index[1].astype(np.int64)
    loops = np.arange(N_NODES, dtype=np.int64)
    src = np.concatenate([src, loops])
    dst = np.concatenate([dst, loops])

    deg = np.bincount(dst, minlength=N_NODES).astype(np.float64)
    dis = (1.0 / np.sqrt(np.maximum(deg, 1.0))).astype(np.float32)

    # degree-sorted deal: rank r -> core r%8, slot r//8
    ranked = np.argsort(-deg, kind="stable")  # node ids by degree desc
    rank_of = np.empty(N_NODES, dtype=np.int64)
    rank_of[ranked] = np.arange(N_NODES)
    core_of = rank_of % N_CORES
    slot_of = rank_of // N_CORES
    # global permuted table row for node n
    grow_of = (core_of * SHARD + slot_of).astype(np.int64)

    e_core = core_of[dst]
    e_slot = slot_of[dst]
    e_gsrc = grow_of[src].astype(np.int32)

    # per (core, tile, row) counts to find D[t] = max over cores+rows
    e_tile = e_slot // P
    e_row = e_slot % P
    # counts[core, tile, row]
    flat = (e_core * SHARD + e_slot).astype(np.int64)
    cnt = np.bincount(flat, minlength=N_CORES * SHARD).reshape(N_CORES, N_TILES, P)
    D = cnt.max(axis=(0, 2)).astype(np.int64)  # [N_TILES]
    D = np.maximum(D, 1)
    doff = np.concatenate([[0], np.cumsum(D)])
    sum_d = int(doff[-1])

    # pack ELL: ell[core][128, sum_d], tile t occupies cols doff[t]:doff[t]+D[t]
    ell = np.full((N_CORES, P, sum_d), PAD_ROW, dtype=np.int32)
    order = np.lexsort((e_slot, e_core))
    oc, ot, orow, ogs = e_core[order], e_tile[order], e_row[order], e_gsrc[order]
    # j-index within (core,slot) groups: order is sorted by (core, slot)
    okey = (oc * SHARD + ot * P + orow)
    uniq, first_idx = np.unique(okey, return_index=True)
    j_idx = np.arange(len(okey)) - np.repeat(first_idx, np.diff(np.concatenate([first_idx, [len(okey)]])))
    ell[oc, orow, doff[ot] + j_idx] = ogs

    # dis in the two layouts the kernels use
    dis_shard = np.zeros((N_CORES, SHARD), dtype=np.float32)
    for c in range(N_CORES):
        ids = ranked[c::N_CORES]
        dis_shard[c, : len(ids)] = dis[ids]
    dis2d = dis_shard.reshape(N_CORES, N_TILES, P).transpose(0, 2, 1)  # [C,128,98]

    return {
        "ranked": ranked,
        "ell": ell,
        "D": [int(d) for d in D],
        "doff": doff,
        "sum_d": sum_d,
        "dis_shard": dis_shard,
        "dis2d": np.ascontiguousarray(dis2d),
    }


# ------------------------------------------------------------- NEFF builders
def _build_neff1():
    """x_shard [SHARD,128] @ W1 [128,16] * dis -> hs1T [16, SHARD]"""
    nc = bacc.Bacc(None, target_bir_lowering=False, debug=True)
    with tile.TileContext(nc) as tc:
        with ExitStack() as ctx:
            dram = ctx.enter_context(tc.tile_pool(name="dram", bufs=1, space="DRAM"))
            x_d = dram.tile([SHARD, F_IN], mybir.dt.float32, kind="ExternalInput", name="x", uniquify=False)
            w1_d = dram.tile([F_IN, HID], mybir.dt.float32, kind="ExternalInput", name="w1", uniquify=False)
            dis_d = dram.tile([P, N_TILES], mybir.dt.float32, kind="ExternalInput", name="dis2d", uniquify=False)
            out_d = dram.tile([HID, SHARD], mybir.dt.float32, kind="ExternalOutput", name="hs1T", uniquify=False)

            sb = ctx.enter_context(tc.tile_pool(name="sb", bufs=3))
            sb1 = ctx.enter_context(tc.tile_pool(name="sb1", bufs=1))
            ps = ctx.enter_context(tc.tile_pool(name="ps", bufs=3, space="PSUM"))

            ident = sb1.tile([P, P], mybir.dt.float32)
            make_identity(nc, ident[:])
            w1_sb = sb1.tile([F_IN, HID], mybir.dt.float32)
            nc.sync.dma_start(out=w1_sb[:], in_=w1_d[:])
            dis_sb = sb1.tile([P, N_TILES], mybir.dt.float32)
            nc.sync.dma_start(out=dis_sb[:], in_=dis_d[:])

            for t in range(N_TILES):
                xt = sb.tile([P, F_IN], mybir.dt.float32, tag="xt")
                nc.sync.dma_start(out=xt[:], in_=x_d[t * P : (t + 1) * P, :])
                xs = sb.tile([P, F_IN], mybir.dt.float32, tag="xs")
                nc.vector.tensor_tensor(
                    out=xs[:],
                    in0=xt[:],
                    in1=dis_sb[:, t : t + 1].to_broadcast([P, F_IN]),
                    op=mybir.AluOpType.mult,
                )
                xT_ps = ps.tile([F_IN, P], mybir.dt.float32, tag="xT")
                nc.tensor.transpose(out=xT_ps[:], in_=xs[:], identity=ident[:])
                xT_sb = sb.tile([F_IN, P], mybir.dt.float32, tag="xTsb")
                nc.vector.tensor_copy(out=xT_sb[:], in_=xT_ps[:])
                hT_ps = ps.tile([HID, P], mybir.dt.float32, tag="hT")
                nc.tensor.matmul(out=hT_ps[:], lhsT=w1_sb[:], rhs=xT_sb[:], start=True, stop=True)
                hsT_sb = sb.tile([HID, P], mybir.dt.float32, tag="hsT")
                nc.vector.tensor_copy(out=hsT_sb[:], in_=hT_ps[:])
                nc.sync.dma_start(out=out_d[:, t * P : (t + 1) * P], in_=hsT_sb[:])
    nc.compile()
    return nc


def _build_agg_neff(D, doff, sum_d, feat, layer):
    """Shared builder for NEFF2 (layer=1, feat=16) and NEFF3 (layer=2, feat=2)."""
    nc = bacc.Bacc(None, target_bir_lowering=False, debug=True)
    fp32 = mybir.dt.float32
    # lanes of `feat` wide accumulate buffer
    WIDE = 128 if layer == 1 else 32
    LANES = WIDE // feat  # 8 or 16
    with tile.TileContext(nc) as tc:
        with ExitStack() as ctx:
            dram = ctx.enter_context(tc.tile_pool(name="dram", bufs=1, space="DRAM"))
            table_d = dram.tile([TABLE_ROWS, feat], fp32, kind="ExternalInput", name="table", uniquify=False)
            ell_d = dram.tile([P, sum_d], mybir.dt.int32, kind="ExternalInput", name="ell", uniquify=False)
            dis_d = dram.tile([P, N_TILES], fp32, kind="ExternalInput", name="dis2d", uniquify=False)
            if layer == 1:
                b_d = dram.tile([P, HID], fp32, kind="ExternalInput", name="b1", uniquify=False)
                w2_d = dram.tile([HID, OUT], fp32, kind="ExternalInput", name="w2", uniquify=False)
                out_d = dram.tile([OUT, SHARD], fp32, kind="ExternalOutput", name="hs2T", uniquify=False)
            else:
                b_d = dram.tile([P, OUT], fp32, kind="ExternalInput", name="b2", uniquify=False)
                out_d = dram.tile([SHARD, OUT], fp32, kind="ExternalOutput", name="out", uniquify=False)

            sb = ctx.enter_context(tc.tile_pool(name="sb", bufs=3))
            sb1 = ctx.enter_context(tc.tile_pool(name="sb1", bufs=1))
            ps = ctx.enter_context(tc.tile_pool(name="ps", bufs=3, space="PSUM"))

            dis_sb = sb1.tile([P, N_TILES], fp32)
            nc.sync.dma_start(out=dis_sb[:], in_=dis_d[:])
            b_sb = sb1.tile([P, feat if layer == 2 else HID], fp32)
            nc.sync.dma_start(out=b_sb[:], in_=b_d[:])
            if layer == 1:
                ident = sb1.tile([P, P], fp32)
                make_identity(nc, ident[:])
                w2_sb = sb1.tile([HID, OUT], fp32)
                nc.sync.dma_start(out=w2_sb[:], in_=w2_d[:])

            for t in range(N_TILES):
                d = D[t]
                o = int(doff[t])
                idx = sb.tile([P, max(D)], mybir.dt.int32, tag="idx")
                nc.sync.dma_start(out=idx[:, :d], in_=ell_d[:, o : o + d])
                acc = sb.tile([P, WIDE], fp32, tag="acc")
                nc.vector.memset(acc[:], 0.0)
                msgw = sb.tile([P, WIDE], fp32, tag="msgw")
                n_groups = math.ceil(d / LANES)
                for g in range(n_groups):
                    lanes = min(LANES, d - g * LANES)
                    if lanes < LANES:
                        nc.vector.memset(msgw[:, lanes * feat :], 0.0)
                    for j in range(lanes):
                        sl = g * LANES + j
                        nc.gpsimd.indirect_dma_start(
                            out=msgw[:, j * feat : (j + 1) * feat],
                            out_offset=None,
                            in_=table_d[:],
                            in_offset=bass.IndirectOffsetOnAxis(ap=idx[:, sl : sl + 1], axis=0),
                        )
                    nc.vector.tensor_tensor(
                        out=acc[:], in0=acc[:], in1=msgw[:], op=mybir.AluOpType.add
                    )
                # fold lanes
                w = WIDE
                while w > feat:
                    w //= 2
                    nc.vector.tensor_tensor(
                        out=acc[:, :w], in0=acc[:, :w], in1=acc[:, w : 2 * w], op=mybir.AluOpType.add
                    )
                agg = acc[:, :feat]
                disb = dis_sb[:, t : t + 1].to_broadcast([P, feat])
                scaled = sb.tile([P, feat], fp32, tag="scaled")
                nc.vector.tensor_tensor(out=scaled[:], in0=agg, in1=disb, op=mybir.AluOpType.mult)
                biased = sb.tile([P, feat], fp32, tag="biased")
                nc.vector.tensor_tensor(
                    out=biased[:], in0=scaled[:], in1=b_sb[:, :feat], op=mybir.AluOpType.add
                )
                if layer == 1:
                    r = sb.tile([P, HID], fp32, tag="relu")
                    nc.scalar.activation(out=r[:], in_=biased[:], func=mybir.ActivationFunctionType.Relu)
                    r2 = sb.tile([P, HID], fp32, tag="r2")
                    nc.vector.tensor_tensor(
                        out=r2[:], in0=r[:], in1=dis_sb[:, t : t + 1].to_broadcast([P, HID]), op=mybir.AluOpType.mult
                    )
                    rT_ps = ps.tile([HID, P], fp32, tag="rT")
                    nc.tensor.transpose(out=rT_ps[:], in_=r2[:], identity=ident[:])
                    rT_sb = sb.tile([HID, P], fp32, tag="rTsb")
                    nc.vector.tensor_copy(out=rT_sb[:], in_=rT_ps[:])
                    h2T_ps = ps.tile([OUT, P], fp32, tag="h2T")
                    nc.tensor.matmul(out=h2T_ps[:], lhsT=w2_sb[:], rhs=rT_sb[:], start=True, stop=True)
                    h2T_sb = sb.tile([OUT, P], fp32, tag="h2Tsb")
                    nc.vector.tensor_copy(out=h2T_sb[:], in_=h2T_ps[:])
                    nc.sync.dma_start(out=out_d[:, t * P : (t + 1) * P], in_=h2T_sb[:])
                else:
                    # log_softmax over the 2 columns
                    mx = sb.tile([P, 1], fp32, tag="mx")
                    nc.vector.tensor_reduce(out=mx[:], in_=biased[:], axis=mybir.AxisListType.X, op=mybir.AluOpType.max)
                    sh = sb.tile([P, OUT], fp32, tag="sh")
                    nc.vector.tensor_tensor(
                        out=sh[:], in0=biased[:], in1=mx[:].to_broadcast([P, OUT]), op=mybir.AluOpType.subtract
                    )
                    ex = sb.tile([P, OUT], fp32, tag="ex")
                    nc.scalar.activation(out=ex[:], in_=sh[:], func=mybir.ActivationFunctionType.Exp)
                    sm = sb.tile([P, 1], fp32, tag="sm")
                    nc.vector.tensor_reduce(out=sm[:], in_=ex[:], axis=mybir.AxisListType.X, op=mybir.AluOpType.add)
                    ls = sb.tile([P, 1], fp32, tag="ls")
                    nc.scalar.activation(out=ls[:], in_=sm[:], func=mybir.ActivationFunctionType.Ln)
                    res = sb.tile([P, OUT], fp32, tag="res")
                    nc.vector.tensor_tensor(
                        out=res[:], in0=sh[:], in1=ls[:].to_broadcast([P, OUT]), op=mybir.AluOpType.subtract
                    )
                    nc.sync.dma_start(out=out_d[t * P : (t + 1) * P, :], in_=res[:])
    nc.compile()
    return nc


# ------------------------------------------------------------------- driver
def kernel(x, edge_index, W1, b1, W2, b2, _profile=False):
    x = np.asarray(x, dtype=np.float32)
    W1 = np.asarray(W1, dtype=np.float32)
    b1 = np.asarray(b1, dtype=np.float32)
    W2 = np.asarray(W2, dtype=np.float32)
    b2 = np.asarray(b2, dtype=np.float32)
    pp = _preprocess(np.asarray(edge_index))
    ranked, ell, D, doff, sum_d = pp["ranked"], pp["ell"], pp["D"], pp["doff"], pp["sum_d"]

    key = ("neffs", tuple(D))
    if key not in _CACHE:
        _CACHE[key] = (
            _build_neff1(),
            _build_agg_neff(D, doff, sum_d, HID, layer=1),
            _build_agg_neff(D, doff, sum_d, OUT, layer=2),
        )
    nc1, nc2, nc3 = _CACHE[key]
    cores = list(range(N_CORES))
    prof = []

    # NEFF1
    in1 = []
    for c in cores:
        ids = ranked[c::N_CORES]
        xs = np.zeros((SHARD, F_IN), dtype=np.float32)
        xs[: len(ids)] = x[ids]
        in1.append({"x": xs, "w1": W1, "dis2d": pp["dis2d"][c]})
    r1 = run_bass_kernel_spmd(nc1, in1, cores, trace=False)
    prof.append(r1)
    hs1 = np.concatenate([r1.results[c]["hs1T"].T for c in cores], axis=0)
    hs1 = np.ascontiguousarray(hs1)  # [TABLE_ROWS, 16]

    # NEFF2
    in2 = [
        {"table": hs1, "ell": ell[c], "dis2d": pp["dis2d"][c], "b1": np.tile(b1[None, :], (128, 1)), "w2": W2}
        for c in cores
    ]
    r2 = run_bass_kernel_spmd(nc2, in2, cores, trace=False)
    prof.append(r2)
    hs2 = np.concatenate([r2.results[c]["hs2T"].T for c in cores], axis=0)
    hs2 = np.ascontiguousarray(hs2)  # [TABLE_ROWS, 2]

    # NEFF3
    in3 = [
        {"table": hs2, "ell": ell[c], "dis2d": pp["dis2d"][c], "b2": np.tile(b2[None, :], (128, 1))}
        for c in cores
    ]
    r3 = run_bass_kernel_spmd(nc3, in3, cores, trace=False)
    prof.append(r3)
    kernel._last_inmaps = (in1, in2, in3)
    kernel._last_ncs = (nc1, nc2, nc3)

    out = np.empty((N_NODES, OUT), dtype=np.float32)
    for c in cores:
        ids = ranked[c::N_CORES]
        out[ids] = r3.results[c]["out"][: len(ids)]
    if _profile:
        kernel._last_profile = prof
    return out

